# revision 1
# baseline (speedup 1.0000x reference)
"""Trainium2 Bass kernel for nn_LinearEncoder (gnn_message_passing).

Reference computes, for N=512 nodes with n_in = n_out = 256:
    i, j = triu_indices(N, k=1)
    edges = concat([x[i], x[j]], -1)            # [E, 512]
    h = edges @ W.T + b                         # [E, 256]
    out[i, j] = h ; out = out + out.T           # [N, N, 256], 0 diagonal

Key algebraic identity: with W = [W1 | W2],
    h(i, j) = A[i] + B[j] + b,   A = x @ W1.T,  B = x @ W2.T
so the full output is
    out[i, j] = A[min(i,j)] + B'[max(i,j)]      (B' = B + b), 0 on diagonal.

Sharding: output rows split across 8 cores (64 rows each), one SPMD
program.  Core k receives x pre-rotated by its row base
(x_rot[t] = x[(base+t) % 512]) so the triangular "diagonal block" sits at
local columns s in [0, 64) on every core; region selection (A vs B')
enters only through small 0/1 mask *inputs*.

Per row-pair rp (rows r0 = 2rp, r0+1), the device computes:
  - three 128-wide column blocks:  PSUM = masks.T @ row-table (bf16 hi+lo
    split, exact to ~2^-17), run CONCURRENTLY on the PE via distinct
    row-groups (tile_position), then evacuated by VectorE as
    sbuf = PSUM + ColTable_f32 (column terms exact fp32);
  - block0 upper half: same masked-broadcast + DVE fold;
  - the triangular diagonal block: two constant masked-selection matmul
    pairs (including the exact-zero diagonal), evacuated by ScalarE.
DMA streams ~33.5 MB/core of output to HBM — the roofline.
"""

import os
import sys

for _p in ("/opt/trn_rl_repo", "/root/.axon_site/_ro/trn_rl_repo"):
    if os.path.isdir(_p) and _p not in sys.path:
        sys.path.insert(0, _p)

import numpy as np
import ml_dtypes

import concourse.bass as bass
import concourse.bacc as bacc
import concourse.mybir as mybir
import concourse.tile as tile
from concourse.bass_utils import run_bass_kernel_spmd

N = 512
CH = 256          # n_out
NIN = 256         # n_in
NCORES = 8
RB = N // NCORES  # 64 rows per core
F32 = mybir.dt.float32
BF16 = mybir.dt.bfloat16
BF16NP = ml_dtypes.bfloat16


# --------------------------------------------------------------------------
# host-side constant builders
# --------------------------------------------------------------------------

def _masks_RL(k: int):
    """R/L region indicators over local columns s for core k."""
    base = RB * k
    wrap = N - base  # columns s >= wrap hold wrapped (j < base) entries
    s = np.arange(N)
    R = ((s >= 64) & (s < wrap)).astype(np.float32)
    L = (s >= wrap).astype(np.float32)
    return R, L


def _diag_consts():
    """Constant masked-selection weights for the 64x64 diagonal blocks.

    For row-pair rp, output column m = q*64 + s (q in {0,1}, s in [0,64)),
    with r_q = 2*rp + q and rhs = [A_rot[0:64] ; B'_rot[0:64]] (K = 128):
      L side (s < r_q):  value = A_rot[s] + B'_rot[r_q]
      R side (s > r_q):  value = B'_rot[s] + A_rot[r_q]
      s == r_q: all weights zero -> exact 0 output.
    """
    dl = np.zeros((128, 32 * 128), np.float32)
    dr = np.zeros((128, 32 * 128), np.float32)
    for rp in range(32):
        for q in range(2):
            r_q = 2 * rp + q
            for s in range(64):
                m = rp * 128 + q * 64 + s
                if s < r_q:
                    dl[s, m] = 1.0            # A_rot[s]
                    dl[64 + r_q, m] = 1.0     # B'_rot[r_q]
                elif s > r_q:
                    dr[64 + s, m] = 1.0       # B'_rot[s]
                    dr[r_q, m] = 1.0          # A_rot[r_q]
    return dl, dr


def _shared_inputs(W: np.ndarray, b: np.ndarray):
    W = np.asarray(W, np.float32)
    b = np.asarray(b, np.float32)
    dl, dr = _diag_consts()
    w12 = np.concatenate(
        [np.ascontiguousarray(W[:, :NIN].T), np.ascontiguousarray(W[:, NIN:].T)],
        axis=1)                                     # [in, 2*out] = [A | B]
    b2 = np.concatenate([np.zeros(CH, np.float32), b]).reshape(1, 2 * CH)
    return {
        "w12t": w12,
        "b2_row": b2,
        "diag_l": dl.astype(BF16NP),
        "diag_r": dr.astype(BF16NP),
    }


def _core_inputs(x: np.ndarray, k: int):
    x = np.asarray(x, np.float32)
    base = RB * k
    x_rot = np.roll(x, -base, axis=0)
    R, L = _masks_RL(k)

    cm = np.zeros((128, 8), np.float32)
    for t in range(4):
        cm[:, t] = R[128 * t:128 * (t + 1)]
        cm[:, 4 + t] = L[128 * t:128 * (t + 1)]

    # One [128, 512] lhsT tensor: the masked-broadcast weights for the
    # three main column blocks live in PE row-groups 0/1/2 (partitions
    # 0-3, 32-35, 64-67; rows = R, L, R, L over hi/lo flat tables) and
    # block0-upper's K=8 weights in row-group 3 (partitions 96-103) —
    # the four small-K matmuls then run concurrently on the PE.
    wm4 = np.stack([R, L, R, L])                        # [4, 512]
    wm0 = np.zeros((8, 128), np.float32)
    p = np.arange(64)
    wm0[0, :64] = R[64 + p]
    wm0[1, :64] = L[64 + p]
    wm0[2, :64] = R[64 + p]
    wm0[3, :64] = L[64 + p]
    wm0[4, 64:] = R[64 + p]
    wm0[5, 64:] = L[64 + p]
    wm0[6, 64:] = R[64 + p]
    wm0[7, 64:] = L[64 + p]
    wmbig = np.zeros((128, 512), np.float32)
    for gp in (0, 32, 64):
        wmbig[gp:gp + 4, :] = wm4
    wmbig[96:104, 0:128] = wm0
    return {
        "xt_rot": np.ascontiguousarray(x_rot.T),  # [in=256, node=512]
        "cm": cm,
        "wm": wmbig.astype(BF16NP),
    }


# --------------------------------------------------------------------------
# device program
# --------------------------------------------------------------------------

_PROGRAM = None


def _build_program() -> bass.Bass:
    nc = bacc.Bacc()
    f32 = F32
    npad = 68  # padded flat scratch rows

    # ---- dram tensors -----------------------------------------------------
    xt_rot = nc.dram_tensor("xt_rot", [NIN, N], f32, kind="ExternalInput")
    w12t = nc.dram_tensor("w12t", [NIN, 2 * CH], f32, kind="ExternalInput")
    b2_row = nc.dram_tensor("b2_row", [1, 2 * CH], f32, kind="ExternalInput")
    cm = nc.dram_tensor("cm", [128, 8], f32, kind="ExternalInput")
    d_wm = nc.dram_tensor("wm", [128, N], BF16, kind="ExternalInput")
    d_dl = nc.dram_tensor("diag_l", [128, 32 * 128], BF16, kind="ExternalInput")
    d_dr = nc.dram_tensor("diag_r", [128, 32 * 128], BF16, kind="ExternalInput")

    # DMA-native contiguous layouts; the host unpicks them (free).
    # slab_m[3g + J-1, p, (sub, q, ch)] = value(row 8g+2sub+q, s = 128J+p)
    # out0d/u[g, q*64+s, (sub, ch)]    = diag/upper block values
    slab_m = nc.dram_tensor("slab_m", [24, 128, 2048], f32,
                            kind="ExternalOutput")
    out0d = nc.dram_tensor("out0d", [8, 128, 1024], f32, kind="ExternalOutput")
    out0u = nc.dram_tensor("out0u", [8, 128, 1024], f32, kind="ExternalOutput")

    with tile.TileContext(nc) as tc:
        with (
            tc.tile_pool(name="const", bufs=1) as cpool,
            tc.tile_pool(name="tmp", bufs=3) as tpool,
            tc.tile_pool(name="psA", bufs=5, space="PSUM") as psA,
            tc.tile_pool(name="ps0", bufs=3, space="PSUM") as ps0,
            tc.tile_pool(name="stM", bufs=7) as stM,
            tc.tile_pool(name="st0", bufs=6) as st0,
        ):
            # ---- load inputs ---------------------------------------------
            def load(dram, shape, dtype, tag):
                t = cpool.tile(shape, dtype, tag=tag)
                nc.sync.dma_start(out=t[:], in_=dram[:])
                return t

            xt0 = load(xt_rot[0:128, :], [128, N], f32, "xt0")
            xt1 = load(xt_rot[128:256, :], [128, N], f32, "xt1")
            w12a = load(w12t[0:128, :], [128, 2 * CH], f32, "w12a")
            w12b = load(w12t[128:256, :], [128, 2 * CH], f32, "w12b")
            b2t = load(b2_row, [1, 2 * CH], f32, "b2t")
            cmt = load(cm, [128, 8], f32, "cmt")
            wmt = load(d_wm, [128, N], BF16, "wmt")
            dlt = cpool.tile([128, 32 * 128], BF16, tag="dlt")
            nc.gpsimd.dma_start(out=dlt[:], in_=d_dl[:])
            drt = cpool.tile([128, 32 * 128], BF16, tag="drt")
            nc.gpsimd.dma_start(out=drt[:], in_=d_dr[:])

            ones1 = cpool.tile([1, 128], f32, tag="ones1")
            nc.vector.memset(ones1[:], 1.0)

            # ---- phase 1: tables [A | B'] (one [128, 512] psum per s) ----
            A_t, Bp_t = [], []
            for s in range(4):
                pa = psA.tile([128, 2 * CH], f32, tag="pj", name=f"ptb{s}")
                mmd = nc.tensor.matmul
                mmd(pa[:], xt0[:, 128 * s:128 * (s + 1)], w12a[:],
                    start=True, stop=False)
                mmd(pa[:], xt1[:, 128 * s:128 * (s + 1)], w12b[:],
                    start=False, stop=False)
                mmd(pa[:], ones1[:], b2t[:], start=False, stop=True)
                comb = cpool.tile([128, 2 * CH], f32, tag=f"AB{s}")
                if s % 2 == 0:
                    nc.vector.tensor_copy(out=comb[:], in_=pa[:])
                else:
                    nc.scalar.copy(out=comb[:], in_=pa[:])
                A_t.append(comb[:, 0:CH])
                Bp_t.append(comb[:, CH:2 * CH])

            # ---- phase 1b: mixed column tables Cmix = R*B' + L*A (f32) ---
            Cmix = []
            for s in range(4):
                eng = nc.vector if s % 2 == 0 else nc.gpsimd
                t1 = tpool.tile([128, CH], f32, tag="t1")
                eng.tensor_scalar(t1[:], Bp_t[s], cmt[:, s:s + 1], None,
                                  mybir.AluOpType.mult)
                t2 = tpool.tile([128, CH], f32, tag="t2")
                eng.tensor_scalar(t2[:], A_t[s], cmt[:, 4 + s:5 + s], None,
                                  mybir.AluOpType.mult)
                cx = cpool.tile([128, CH], f32, tag=f"C{s}")
                eng.tensor_tensor(cx[:], t1[:], t2[:], mybir.AluOpType.add)
                Cmix.append(cx)

            # duplicated f32 column tables for the r-paired main tiles
            CD = {}
            for s in (1, 2, 3):
                dup = cpool.tile([128, 2 * CH], f32, tag=f"CD{s}")
                nc.vector.tensor_copy(out=dup[:, 0:CH], in_=Cmix[s][:])
                nc.scalar.copy(out=dup[:, CH:2 * CH], in_=Cmix[s][:])
                CD[s] = dup


            def hi_lo(src_ap, tag):
                """split a f32 [128, W] AP into bf16 hi + lo tiles."""
                wdt = src_ap.shape[-1]
                hi = cpool.tile([128, wdt], BF16, tag=f"{tag}h")
                nc.scalar.copy(out=hi[:], in_=src_ap)
                h32 = tpool.tile([128, wdt], f32, tag="h32")
                nc.scalar.copy(out=h32[:], in_=hi[:])
                d = tpool.tile([128, wdt], f32, tag="d32")
                nc.vector.tensor_sub(d[:], src_ap, h32[:])
                lo = cpool.tile([128, wdt], BF16, tag=f"{tag}l")
                nc.vector.tensor_copy(out=lo[:], in_=d[:])
                return hi, lo

            ah, al = hi_lo(A_t[0], "a0")
            bh, bl = hi_lo(Bp_t[0], "b0")
            # block0-upper f32 column table, replicated to both q-halves
            cup = cpool.tile([128, CH], f32, tag="cup")
            nc.gpsimd.dma_start(out=cup[0:64, :], in_=Cmix[0][64:128, :])
            nc.gpsimd.dma_start(out=cup[64:128, :], in_=Cmix[0][64:128, :])
            # diag combined rhs [A_rot[0:64] ; B'_rot[0:64]] (hi / lo)
            dcb_h = cpool.tile([128, CH], BF16, tag="dcbh")
            dcb_l = cpool.tile([128, CH], BF16, tag="dcbl")
            nc.vector.tensor_copy(out=dcb_h[0:64, :], in_=ah[0:64, :])
            nc.vector.tensor_copy(out=dcb_l[0:64, :], in_=al[0:64, :])
            nc.gpsimd.dma_start(out=dcb_h[64:128, :], in_=bh[0:64, :])
            nc.gpsimd.dma_start(out=dcb_l[64:128, :], in_=bl[0:64, :])
            # flat row tables: direct SBUF->SBUF flatten into partitions
            # 0-7, then replicated to partition groups 32/64/96 (walrus
            # requires rhs to start at the same partition as the weights).
            rp4 = cpool.tile([104, 64 * CH], BF16, tag="rp4")
            nc.vector.memset(rp4[0:8, 63 * CH:64 * CH], 0.0)
            for i, t in enumerate((ah, bh, al, bl)):
                nc.gpsimd.dma_start(out=rp4[i:i + 1, :], in_=t[0:64, :])
                nc.gpsimd.dma_start(out=rp4[4 + i:5 + i, 0:63 * CH],
                                  in_=t[1:64, :])
            for gp in (32, 64, 96):
                nc.gpsimd.dma_start(out=rp4[gp:gp + 8, :], in_=rp4[0:8, :])

            # ---- phase 2: main loop --------------------------------------
            for g in range(8):
                sM = {J: stM.tile([128, 4 * 512], f32, tag="sm",
                                  name=f"sm_{g}_{J}")
                      for J in (1, 2, 3)}
                s0d = st0.tile([128, 4 * CH], f32, tag="s0")
                s0u = st0.tile([128, 4 * CH], f32, tag="s0")
                for sub in range(4):
                    rp = 4 * g + sub
                    off = 2 * rp * CH
                    # four small-K masked-broadcast matmuls in distinct PE
                    # row-groups -> concurrent execution.
                    pj = {}
                    for J in (1, 2, 3):
                        gp = 32 * (J - 1)
                        p = psA.tile([128, 512], f32, tag="pj",
                                     name=f"pj_{rp}_{J}")
                        nc.tensor.matmul(
                            p[:], wmt[gp:gp + 4, 128 * J:128 * (J + 1)],
                            rp4[gp:gp + 4, off:off + 512],
                            start=True, stop=True, tile_position=(gp, 0))
                        pj[J] = p
                    pu = ps0.tile([128, CH], f32, tag="p0", name=f"pu_{rp}")
                    nc.tensor.matmul(
                        pu[:], wmt[96:104, 0:128],
                        rp4[96:104, off:off + CH],
                        start=True, stop=True, tile_position=(96, 0))
                    # diagonal block (s in [0,64)), rows r0, r0+1
                    pd = ps0.tile([128, CH], f32, tag="p0", name=f"pd_{rp}")
                    dl_sl = dlt[:, 128 * rp:128 * (rp + 1)]
                    dr_sl = drt[:, 128 * rp:128 * (rp + 1)]
                    nc.tensor.matmul(pd[:], dl_sl, dcb_h[:],
                                     start=True, stop=False)
                    nc.tensor.matmul(pd[:], dl_sl, dcb_l[:],
                                     start=False, stop=False)
                    nc.tensor.matmul(pd[:], dr_sl, dcb_h[:],
                                     start=False, stop=False)
                    nc.tensor.matmul(pd[:], dr_sl, dcb_l[:],
                                     start=False, stop=True)
                    # evacuation: VectorE folds the f32 column tables in;
                    # ScalarE evacuates the diagonal block.
                    for J in (1, 2, 3):
                        nc.vector.tensor_add(
                            sM[J][:, 512 * sub:512 * (sub + 1)],
                            pj[J][:], CD[J][:])
                    nc.vector.tensor_add(
                        s0u[:, CH * sub:CH * (sub + 1)], pu[:], cup[:])
                    nc.scalar.copy(out=s0d[:, CH * sub:CH * (sub + 1)],
                                   in_=pd[:])
                    if sub in (1, 3):
                        h = (sub - 1) // 2
                        hs, he = 1024 * h, 1024 * (h + 1)
                        for J in (1, 2):
                            nc.sync.dma_start(
                                out=slab_m[3 * g + J - 1][:, hs:he],
                                in_=sM[J][:, hs:he])
                        nc.scalar.dma_start(
                            out=slab_m[3 * g + 2][:, hs:he],
                            in_=sM[3][:, hs:he])
                        nc.scalar.dma_start(
                            out=out0u[g][:, 512 * h:512 * (h + 1)],
                            in_=s0u[:, 512 * h:512 * (h + 1)])
                        nc.scalar.dma_start(
                            out=out0d[g][:, 512 * h:512 * (h + 1)],
                            in_=s0d[:, 512 * h:512 * (h + 1)])

    nc.compile()
    return nc


def _program() -> bass.Bass:
    global _PROGRAM
    if _PROGRAM is None:
        _PROGRAM = _build_program()
    return _PROGRAM


# --------------------------------------------------------------------------
# host entry point
# --------------------------------------------------------------------------

def _assemble(results):
    """8 per-core result dicts -> full [512, 512, 256] output."""
    out = np.empty((N, N, CH), np.float32)
    for k in range(NCORES):
        r = results[k]
        slab = np.empty((RB, N, CH), np.float32)
        # out0d/u: [g, q*64+s, (sub, ch)] -> rows 8g+2sub+q, cols s / 64+s
        d = np.asarray(r["out0d"]).reshape(8, 2, 64, 4, CH)
        slab[:, 0:64, :] = d.transpose(0, 3, 1, 2, 4).reshape(RB, 64, CH)
        u = np.asarray(r["out0u"]).reshape(8, 2, 64, 4, CH)
        slab[:, 64:128, :] = u.transpose(0, 3, 1, 2, 4).reshape(RB, 64, CH)
        # slab_m: [3g+J-1, p, (sub, q, ch)] -> rows 8g+2sub+q, col 128J+p
        m = np.asarray(r["slab_m"]).reshape(8, 3, 128, 4, 2, CH)
        slab[:, 128:512, :] = (
            m.transpose(0, 3, 4, 1, 2, 5).reshape(RB, 384, CH))
        base = RB * k
        out[base:base + RB] = np.roll(slab, base, axis=1)
    return out


def build_in_maps(x, W, b):
    shared = _shared_inputs(W, b)
    return [dict(shared, **_core_inputs(x, k)) for k in range(NCORES)]


def kernel(x, W, b):
    nc = _program()
    in_maps = build_in_maps(x, W, b)
    res = run_bass_kernel_spmd(nc, in_maps, core_ids=list(range(NCORES)))
    return _assemble(res.results)



# revision 3
# speedup vs baseline: 1.9027x; 1.9027x over previous
"""Trainium2 Bass kernel for nn_LinearEncoder (gnn_message_passing), v2.

Reference, for N=512 nodes, n_in = n_out = 256:
    i, j = triu_indices(N, k=1)
    h = concat([x[i], x[j]]) @ W.T + b        # [E, 256]
    out[i, j] = h ; out = out + out.T         # [512, 512, 256], 0 diagonal

Algebraic identity (W = [W1 | W2]):  out[i, j] = A[min] + B'[max],
    A = x @ W1.T,  B' = x @ W2.T + b,  zero diagonal.

v2 exploits the output symmetry: each unordered pair {r, j} is computed
on exactly ONE core, as bf16, and the host mirrors it into both [r, j]
and [j, r].  Pair assignment: row r owns circular distances d = 1..255
(plus d = 256 for r < 256), so core k (rows [64k, 64k+64)) computes the
rotated-column rectangle s in [0, 320), cols j = (64k + s) % 512 — a 20%
padded cover of its distance band.  Out bytes/core: 10.5 MB (vs 33.5 f32
full-matrix) — the DMA roofline at ~390 GB/s is ~27 us.

Device program (partition dim = output channel):
  - PE: tables AT[ch, s], BpT[ch, s] = (W1 @ xT_rot), (W2 @ xT_rot + b)
    for s in [0, 320), x split hi/lo bf16 (exact to ~2^-17).
  - region masks (input): column term M = AT + R*(BpT - AT),
    per-row bias BSEG[ch, (j, t)] = BpT - R_j*(BpT - AT) (R_j = 1 iff
    64-col segment j is unwrapped, i.e. global col > global row).
  - DVE/Pool: slab[ch, (t, j, s)] = M[ch, (j, s)] + BSEG[ch, (j, t)]
    via one broadcast tensor_tensor per 8-row chunk, bf16 out.
  - sync queue streams h=0 chunks, scalar queue h=1 chunks to HBM.
"""

import os
import sys

for _p in ("/opt/trn_rl_repo", "/root/.axon_site/_ro/trn_rl_repo"):
    if os.path.isdir(_p) and _p not in sys.path:
        sys.path.insert(0, _p)

import numpy as np
import ml_dtypes

import concourse.bass as bass
import concourse.bacc as bacc
import concourse.mybir as mybir
import concourse.tile as tile
from concourse.bass_utils import run_bass_kernel_spmd

N = 512
CH = 256          # n_out
NIN = 256         # n_in
NCORES = 8
RB = N // NCORES  # 64 rows per core
SCOL = 320        # rotated-column rectangle width
NSEG = 5          # 64-wide column segments
F32 = mybir.dt.float32
BF16 = mybir.dt.bfloat16
BF16NP = ml_dtypes.bfloat16


# --------------------------------------------------------------------------
# host-side input builders
# --------------------------------------------------------------------------

def _shared_inputs(W, b):
    W = np.asarray(W, np.float32)
    b = np.asarray(b, np.float32)
    wa = np.ascontiguousarray(W[:, :NIN].T)       # [f, ch]
    wb = np.ascontiguousarray(W[:, NIN:].T)
    return {
        "wa": wa.astype(BF16NP),
        "wb": wb.astype(BF16NP),
        "bcol": b.reshape(1, CH).astype(BF16NP),
    }


def _core_inputs(x, k):
    x = np.asarray(x, np.float32)
    base = RB * k
    idx = (base + np.arange(SCOL)) % N
    xr = np.ascontiguousarray(x.T[:, idx])        # [f, s] rotated
    xh = xr.astype(BF16NP)
    xl = (xr - xh.astype(np.float32)).astype(BF16NP)

    wrap = N - base
    rl = np.zeros((128, 3 * NSEG), np.float32)
    for j in range(NSEG):
        r = 1.0 if 64 * (j + 1) <= wrap else 0.0
        rl[:, j] = r
        rl[:, NSEG + j] = 1.0 - r
        rl[:, 2 * NSEG + j] = -r
    return {"xh": xh, "xl": xl, "rl": rl}


# --------------------------------------------------------------------------
# device program
# --------------------------------------------------------------------------

_PROGRAM = None


def _build_program() -> bass.Bass:
    nc = bacc.Bacc()
    f32 = F32
    AL = mybir.AluOpType

    d_xh = nc.dram_tensor("xh", [NIN, SCOL], BF16, kind="ExternalInput")
    d_xl = nc.dram_tensor("xl", [NIN, SCOL], BF16, kind="ExternalInput")
    d_wa = nc.dram_tensor("wa", [NIN, CH], BF16, kind="ExternalInput")
    d_wb = nc.dram_tensor("wb", [NIN, CH], BF16, kind="ExternalInput")
    d_bcol = nc.dram_tensor("bcol", [1, CH], BF16, kind="ExternalInput")
    d_rl = nc.dram_tensor("rl", [128, 3 * NSEG], f32, kind="ExternalInput")

    # outp[h][p, (c, t, j, s)]: ch = 128h + p, row t' = 8c + t,
    # rotated col = 64j + s.
    d_out = nc.dram_tensor("outp", [2, 128, RB * SCOL], BF16,
                           kind="ExternalOutput")

    with tile.TileContext(nc) as tc:
        with (
            tc.tile_pool(name="const", bufs=1) as cpool,
            tc.tile_pool(name="ps", bufs=4, space="PSUM") as ps,
            tc.tile_pool(name="slab0", bufs=4) as sp0,
            tc.tile_pool(name="slab1", bufs=4) as sp1,
        ):
            def load(dram, shape, dtype, tag, eng=None):
                t = cpool.tile(shape, dtype, tag=tag, name=tag)
                (eng or nc.sync).dma_start(out=t[:], in_=dram)
                return t

            xh0 = load(d_xh[0:128, :], [128, SCOL], BF16, "xh0")
            xh1 = load(d_xh[128:256, :], [128, SCOL], BF16, "xh1")
            xl0 = load(d_xl[0:128, :], [128, SCOL], BF16, "xl0", nc.scalar)
            xl1 = load(d_xl[128:256, :], [128, SCOL], BF16, "xl1", nc.scalar)
            wa0 = load(d_wa[0:128, :], [128, CH], BF16, "wa0")
            wa1 = load(d_wa[128:256, :], [128, CH], BF16, "wa1")
            wb0 = load(d_wb[0:128, :], [128, CH], BF16, "wb0", nc.scalar)
            wb1 = load(d_wb[128:256, :], [128, CH], BF16, "wb1", nc.scalar)
            bcol = load(d_bcol[:], [1, CH], BF16, "bcol")
            rlt = load(d_rl[:], [128, 3 * NSEG], f32, "rlt")

            ones = cpool.tile([1, SCOL], BF16, tag="ones", name="ones")
            nc.vector.memset(ones[:], 1.0)

            # ---- tables ----------------------------------------------------
            AT, BpT, dT, M, BSEG = {}, {}, {}, {}, {}
            for h in (0, 1):
                cs = slice(128 * h, 128 * (h + 1))
                for nm, w0, w1, with_b in (("A", wa0, wa1, False),
                                           ("B", wb0, wb1, True)):
                    p = ps.tile([128, SCOL], f32, tag="pt", name=f"pt{nm}{h}")
                    mm = nc.tensor.matmul
                    mm(p[:], w0[:, cs], xh0[:], start=True, stop=False)
                    mm(p[:], w1[:, cs], xh1[:], start=False, stop=False)
                    mm(p[:], w0[:, cs], xl0[:], start=False, stop=False)
                    mm(p[:], w1[:, cs], xl1[:], start=False,
                       stop=not with_b)
                    if with_b:
                        mm(p[:], bcol[0:1, cs], ones[:], start=False,
                           stop=True)
                    t = cpool.tile([128, SCOL], f32, tag=f"T{nm}{h}",
                                   name=f"T{nm}{h}")
                    nc.scalar.copy(out=t[:], in_=p[:])
                    (AT if nm == "A" else BpT)[h] = t

                d = cpool.tile([128, SCOL], f32, tag=f"d{h}", name=f"d{h}")
                nc.vector.tensor_sub(d[:], BpT[h][:], AT[h][:])
                dT[h] = d

                # M = AT + R*d (col term); BSEG[(j, t)] = BpT - R_j*d (bias)
                m = cpool.tile([128, SCOL], f32, tag=f"M{h}", name=f"M{h}")
                bs = cpool.tile([128, NSEG * RB], f32, tag=f"BS{h}",
                                name=f"BS{h}")
                for j in range(NSEG):
                    sl = slice(64 * j, 64 * (j + 1))
                    nc.vector.scalar_tensor_tensor(
                        out=m[:, sl], in0=d[:, sl],
                        scalar=rlt[:, j:j + 1], in1=AT[h][:, sl],
                        op0=AL.mult, op1=AL.add)
                    nc.vector.scalar_tensor_tensor(
                        out=bs[:, RB * j:RB * (j + 1)], in0=d[:, 0:RB],
                        scalar=rlt[:, 2 * NSEG + j:2 * NSEG + j + 1],
                        in1=BpT[h][:, 0:RB], op0=AL.mult, op1=AL.add)
                M[h], BSEG[h] = m, bs

            # ---- slabs -----------------------------------------------------
            for c in range(8):
                for h in (0, 1):
                    pool = sp0 if h == 0 else sp1
                    slab = pool.tile([128, 8 * SCOL], BF16, tag="sl",
                                     name=f"sl{c}_{h}")
                    out_ap = slab[:].rearrange(
                        "p (t j s) -> p t j s", t=8, j=NSEG, s=64)
                    m_ap = (M[h][:].rearrange("p (j s) -> p j s", j=NSEG)
                            .unsqueeze(1).broadcast_to([128, 8, NSEG, 64]))
                    b_ap = (BSEG[h][:]
                            .rearrange("p (j t) -> p j t", j=NSEG)
                            [:, :, 8 * c:8 * (c + 1)]
                            .transpose([0, 2, 1]).unsqueeze(3)
                            .broadcast_to([128, 8, NSEG, 64]))
                    eng = nc.vector if (c + h) % 2 == 0 else nc.gpsimd
                    eng.tensor_tensor(out_ap, m_ap, b_ap, AL.add)
                    q = nc.sync if h == 0 else nc.scalar
                    q.dma_start(
                        out=d_out[h][:, 8 * SCOL * c:8 * SCOL * (c + 1)],
                        in_=slab[:])

    nc.compile()
    return nc


def _program() -> bass.Bass:
    global _PROGRAM
    if _PROGRAM is None:
        _PROGRAM = _build_program()
    return _PROGRAM


# --------------------------------------------------------------------------
# host entry point
# --------------------------------------------------------------------------

_IDX = {}


def _band_idx(dmax):
    """(t_idx, s_idx) of rectangle entries with 1 <= s - t <= dmax."""
    if dmax not in _IDX:
        t, s = np.mgrid[0:RB, 0:SCOL]
        m = (s - t >= 1) & (s - t <= dmax)
        _IDX[dmax] = (t[m], s[m])
    return _IDX[dmax]


def _assemble(results):
    out = np.zeros((N * N, CH), np.float32)
    for k in range(NCORES):
        base = RB * k
        v = np.asarray(results[k]["outp"]).astype(np.float32)
        slab = (v.reshape(2, 128, 8, 8, NSEG, 64)
                .transpose(2, 3, 4, 5, 0, 1).reshape(RB, SCOL, CH))
        t_idx, s_idx = _band_idx(256 if k < 4 else 255)
        r_idx = base + t_idx
        j_idx = (base + s_idx) % N
        vals = slab[t_idx, s_idx]
        out[r_idx * N + j_idx] = vals
        out[j_idx * N + r_idx] = vals
    return out.reshape(N, N, CH)


def build_in_maps(x, W, b):
    shared = _shared_inputs(W, b)
    return [dict(shared, **_core_inputs(x, k)) for k in range(NCORES)]


def kernel(x, W, b):
    nc = _program()
    in_maps = build_in_maps(x, W, b)
    res = run_bass_kernel_spmd(nc, in_maps, core_ids=list(range(NCORES)))
    return _assemble(res.results)


# revision 4
# speedup vs baseline: 1.9731x; 1.0370x over previous
"""Trainium2 Bass kernel for nn_LinearEncoder (gnn_message_passing), v2.

Reference, for N=512 nodes, n_in = n_out = 256:
    i, j = triu_indices(N, k=1)
    h = concat([x[i], x[j]]) @ W.T + b        # [E, 256]
    out[i, j] = h ; out = out + out.T         # [512, 512, 256], 0 diagonal

Algebraic identity (W = [W1 | W2]):  out[i, j] = A[min] + B'[max],
    A = x @ W1.T,  B' = x @ W2.T + b,  zero diagonal.

v2 exploits the output symmetry: each unordered pair {r, j} is computed
on exactly ONE core, as bf16, and the host mirrors it into both [r, j]
and [j, r].  Pair assignment: row r owns circular distances d = 1..255
(plus d = 256 for r < 256), so core k (rows [64k, 64k+64)) computes the
rotated-column rectangle s in [0, 320), cols j = (64k + s) % 512 — a 20%
padded cover of its distance band.  Out bytes/core: 10.5 MB (vs 33.5 f32
full-matrix) — the DMA roofline at ~390 GB/s is ~27 us.

Device program (partition dim = output channel):
  - PE: tables AT[ch, s], BpT[ch, s] = (W1 @ xT_rot), (W2 @ xT_rot + b)
    for s in [0, 320), x split hi/lo bf16 (exact to ~2^-17).
  - region masks (input): column term M = AT + R*(BpT - AT),
    per-row bias BSEG[ch, (j, t)] = BpT - R_j*(BpT - AT) (R_j = 1 iff
    64-col segment j is unwrapped, i.e. global col > global row).
  - DVE/Pool: slab[ch, (t, j, s)] = M[ch, (j, s)] + BSEG[ch, (j, t)]
    via one broadcast tensor_tensor per 8-row chunk, bf16 out.
  - sync queue streams h=0 chunks, scalar queue h=1 chunks to HBM.
"""

import os
import sys

for _p in ("/opt/trn_rl_repo", "/root/.axon_site/_ro/trn_rl_repo"):
    if os.path.isdir(_p) and _p not in sys.path:
        sys.path.insert(0, _p)

import numpy as np
import ml_dtypes

import concourse.bass as bass
import concourse.bacc as bacc
import concourse.mybir as mybir
import concourse.tile as tile
from concourse.bass_utils import run_bass_kernel_spmd

N = 512
CH = 256          # n_out
NIN = 256         # n_in
NCORES = 8
RB = N // NCORES  # 64 rows per core
SCOL = 320        # rotated-column rectangle width
NSEG = 5          # 64-wide column segments
F32 = mybir.dt.float32
BF16 = mybir.dt.bfloat16
BF16NP = ml_dtypes.bfloat16


# --------------------------------------------------------------------------
# host-side input builders
# --------------------------------------------------------------------------

def _shared_inputs(W, b):
    W = np.asarray(W, np.float32)
    b = np.asarray(b, np.float32)
    wa = np.ascontiguousarray(W[:, :NIN].T)       # [f, ch]
    wb = np.ascontiguousarray(W[:, NIN:].T)
    return {
        "wa": wa.astype(BF16NP),
        "wb": wb.astype(BF16NP),
        "bcol": b.reshape(1, CH).astype(BF16NP),
    }


def _core_inputs(x, k):
    x = np.asarray(x, np.float32)
    base = RB * k
    idx = (base + np.arange(SCOL)) % N
    xr = np.ascontiguousarray(x.T[:, idx])        # [f, s] rotated
    xh = xr.astype(BF16NP)
    xl = (xr - xh.astype(np.float32)).astype(BF16NP)

    wrap = N - base
    rl = np.zeros((128, 3 * NSEG), np.float32)
    for j in range(NSEG):
        r = 1.0 if 64 * (j + 1) <= wrap else 0.0
        rl[:, j] = r
        rl[:, NSEG + j] = 1.0 - r
        rl[:, 2 * NSEG + j] = -r
    return {"xh": xh, "xl": xl, "rl": rl}


# --------------------------------------------------------------------------
# device program
# --------------------------------------------------------------------------

_PROGRAM = None


def _build_program() -> bass.Bass:
    nc = bacc.Bacc()
    f32 = F32
    AL = mybir.AluOpType

    d_xh = nc.dram_tensor("xh", [NIN, SCOL], BF16, kind="ExternalInput")
    d_xl = nc.dram_tensor("xl", [NIN, SCOL], BF16, kind="ExternalInput")
    d_wa = nc.dram_tensor("wa", [NIN, CH], BF16, kind="ExternalInput")
    d_wb = nc.dram_tensor("wb", [NIN, CH], BF16, kind="ExternalInput")
    d_bcol = nc.dram_tensor("bcol", [1, CH], BF16, kind="ExternalInput")
    d_rl = nc.dram_tensor("rl", [128, 3 * NSEG], f32, kind="ExternalInput")

    # outp[h][p, (c, t, j, s)]: ch = 128h + p, row t' = 8c + t,
    # rotated col = 64j + s.
    d_out = nc.dram_tensor("outp", [2, 128, RB * SCOL], BF16,
                           kind="ExternalOutput")

    with tile.TileContext(nc) as tc:
        with (
            tc.tile_pool(name="const", bufs=1) as cpool,
            tc.tile_pool(name="ps", bufs=4, space="PSUM") as ps,
            tc.tile_pool(name="slab0", bufs=4) as sp0,
            tc.tile_pool(name="slab1", bufs=4) as sp1,
        ):
            def load(dram, shape, dtype, tag, eng=None):
                t = cpool.tile(shape, dtype, tag=tag, name=tag)
                (eng or nc.sync).dma_start(out=t[:], in_=dram)
                return t

            xh0 = load(d_xh[0:128, :], [128, SCOL], BF16, "xh0")
            xh1 = load(d_xh[128:256, :], [128, SCOL], BF16, "xh1")
            xl0 = load(d_xl[0:128, :], [128, SCOL], BF16, "xl0", nc.scalar)
            xl1 = load(d_xl[128:256, :], [128, SCOL], BF16, "xl1", nc.scalar)
            wa0 = load(d_wa[0:128, :], [128, CH], BF16, "wa0")
            wa1 = load(d_wa[128:256, :], [128, CH], BF16, "wa1")
            wb0 = load(d_wb[0:128, :], [128, CH], BF16, "wb0", nc.scalar)
            wb1 = load(d_wb[128:256, :], [128, CH], BF16, "wb1", nc.scalar)
            bcol = load(d_bcol[:], [1, CH], BF16, "bcol")
            rlt = load(d_rl[:], [128, 3 * NSEG], f32, "rlt")

            ones = cpool.tile([1, SCOL], BF16, tag="ones", name="ones")
            nc.vector.memset(ones[:], 1.0)

            # ---- tables ----------------------------------------------------
            AT, BpT, dT, M, BSEG = {}, {}, {}, {}, {}
            for h in (0, 1):
                cs = slice(128 * h, 128 * (h + 1))
                for nm, w0, w1, with_b in (("A", wa0, wa1, False),
                                           ("B", wb0, wb1, True)):
                    p = ps.tile([128, SCOL], f32, tag="pt", name=f"pt{nm}{h}")
                    mm = nc.tensor.matmul
                    mm(p[:], w0[:, cs], xh0[:], start=True, stop=False)
                    mm(p[:], w1[:, cs], xh1[:], start=False, stop=False)
                    mm(p[:], w0[:, cs], xl0[:], start=False, stop=False)
                    mm(p[:], w1[:, cs], xl1[:], start=False,
                       stop=not with_b)
                    if with_b:
                        mm(p[:], bcol[0:1, cs], ones[:], start=False,
                           stop=True)
                    t = cpool.tile([128, SCOL], f32, tag=f"T{nm}{h}",
                                   name=f"T{nm}{h}")
                    nc.scalar.copy(out=t[:], in_=p[:])
                    (AT if nm == "A" else BpT)[h] = t

                d = cpool.tile([128, SCOL], f32, tag=f"d{h}", name=f"d{h}")
                nc.vector.tensor_sub(d[:], BpT[h][:], AT[h][:])
                dT[h] = d

                # M = AT + R*d (col term); BSEG[(j, t)] = BpT - R_j*d (bias)
                # bf16 so the big adds below run in the DVE 2x perf mode.
                m = cpool.tile([128, SCOL], BF16, tag=f"M{h}", name=f"M{h}")
                bs = cpool.tile([128, NSEG * RB], BF16, tag=f"BS{h}",
                                name=f"BS{h}")
                for j in range(NSEG):
                    sl = slice(64 * j, 64 * (j + 1))
                    nc.vector.scalar_tensor_tensor(
                        out=m[:, sl], in0=d[:, sl],
                        scalar=rlt[:, j:j + 1], in1=AT[h][:, sl],
                        op0=AL.mult, op1=AL.add)
                    nc.vector.scalar_tensor_tensor(
                        out=bs[:, RB * j:RB * (j + 1)], in0=d[:, 0:RB],
                        scalar=rlt[:, 2 * NSEG + j:2 * NSEG + j + 1],
                        in1=BpT[h][:, 0:RB], op0=AL.mult, op1=AL.add)
                M[h], BSEG[h] = m, bs

            # ---- slabs -----------------------------------------------------
            # 5 of 16 chunk-units on GpSimd (slower), none in the first
            # DMA pair of either queue.
            GP = {(2, 1), (3, 0), (4, 1), (5, 0), (6, 1)}
            for cp in range(4):
                for h in (0, 1):
                    pool = sp0 if h == 0 else sp1
                    slab = pool.tile([128, 16 * SCOL], BF16, tag="sl",
                                     name=f"sl{cp}_{h}")
                    for half in (0, 1):
                        c = 2 * cp + half
                        out_ap = slab[:, 8 * SCOL * half:
                                      8 * SCOL * (half + 1)].rearrange(
                            "p (t j s) -> p t j s", t=8, j=NSEG, s=64)
                        m_ap = (M[h][:]
                                .rearrange("p (j s) -> p j s", j=NSEG)
                                .unsqueeze(1)
                                .broadcast_to([128, 8, NSEG, 64]))
                        b_ap = (BSEG[h][:]
                                .rearrange("p (j t) -> p j t", j=NSEG)
                                [:, :, 8 * c:8 * (c + 1)]
                                .transpose([0, 2, 1]).unsqueeze(3)
                                .broadcast_to([128, 8, NSEG, 64]))
                        eng = nc.gpsimd if (c, h) in GP else nc.vector
                        eng.tensor_tensor(out_ap, m_ap, b_ap, AL.add)
                    q = nc.sync if h == 0 else nc.scalar
                    q.dma_start(
                        out=d_out[h][:, 16 * SCOL * cp:16 * SCOL * (cp + 1)],
                        in_=slab[:])

    nc.compile()
    return nc


def _program() -> bass.Bass:
    global _PROGRAM
    if _PROGRAM is None:
        _PROGRAM = _build_program()
    return _PROGRAM


# --------------------------------------------------------------------------
# host entry point
# --------------------------------------------------------------------------

_IDX = {}


def _band_idx(dmax):
    """(t_idx, s_idx) of rectangle entries with 1 <= s - t <= dmax."""
    if dmax not in _IDX:
        t, s = np.mgrid[0:RB, 0:SCOL]
        m = (s - t >= 1) & (s - t <= dmax)
        _IDX[dmax] = (t[m], s[m])
    return _IDX[dmax]


def _assemble(results):
    out = np.zeros((N * N, CH), np.float32)
    for k in range(NCORES):
        base = RB * k
        v = np.asarray(results[k]["outp"]).astype(np.float32)
        slab = (v.reshape(2, 128, 8, 8, NSEG, 64)
                .transpose(2, 3, 4, 5, 0, 1).reshape(RB, SCOL, CH))
        t_idx, s_idx = _band_idx(256 if k < 4 else 255)
        r_idx = base + t_idx
        j_idx = (base + s_idx) % N
        vals = slab[t_idx, s_idx]
        out[r_idx * N + j_idx] = vals
        out[j_idx * N + r_idx] = vals
    return out.reshape(N, N, CH)


def build_in_maps(x, W, b):
    shared = _shared_inputs(W, b)
    return [dict(shared, **_core_inputs(x, k)) for k in range(NCORES)]


def kernel(x, W, b):
    nc = _program()
    in_maps = build_in_maps(x, W, b)
    res = run_bass_kernel_spmd(nc, in_maps, core_ids=list(range(NCORES)))
    return _assemble(res.results)


# revision 6
# speedup vs baseline: 2.9824x; 1.5115x over previous
"""Trainium2 Bass kernel for nn_LinearEncoder (gnn_message_passing), v2.

Reference, for N=512 nodes, n_in = n_out = 256:
    i, j = triu_indices(N, k=1)
    h = concat([x[i], x[j]]) @ W.T + b        # [E, 256]
    out[i, j] = h ; out = out + out.T         # [512, 512, 256], 0 diagonal

Algebraic identity (W = [W1 | W2]):  out[i, j] = A[min] + B'[max],
    A = x @ W1.T,  B' = x @ W2.T + b,  zero diagonal.

v2 exploits the output symmetry: each unordered pair {r, j} is computed
on exactly ONE core, as bf16, and the host mirrors it into both [r, j]
and [j, r].  Pair assignment: row r owns circular distances d = 1..255
(plus d = 256 for r < 256), so core k (rows [64k, 64k+64)) computes the
rotated-column rectangle s in [0, 320), cols j = (64k + s) % 512 — a 20%
padded cover of its distance band.  Out bytes/core: 10.5 MB (vs 33.5 f32
full-matrix) — the DMA roofline at ~390 GB/s is ~27 us.

Device program (partition dim = output channel):
  - PE: tables AT[ch, s], BpT[ch, s] = (W1 @ xT_rot), (W2 @ xT_rot + b)
    for s in [0, 320), x split hi/lo bf16 (exact to ~2^-17).
  - region masks (input): column term M = AT + R*(BpT - AT),
    per-row bias BSEG[ch, (j, t)] = BpT - R_j*(BpT - AT) (R_j = 1 iff
    64-col segment j is unwrapped, i.e. global col > global row).
  - DVE/Pool: slab[ch, (t, j, s)] = M[ch, (j, s)] + BSEG[ch, (j, t)]
    via one broadcast tensor_tensor per 8-row chunk, bf16 out.
  - sync queue streams h=0 chunks, scalar queue h=1 chunks to HBM.
"""

import os
import sys

for _p in ("/opt/trn_rl_repo", "/root/.axon_site/_ro/trn_rl_repo"):
    if os.path.isdir(_p) and _p not in sys.path:
        sys.path.insert(0, _p)

import numpy as np
import ml_dtypes

import concourse.bass as bass
import concourse.bacc as bacc
import concourse.mybir as mybir
import concourse.tile as tile
from concourse.bass_utils import run_bass_kernel_spmd

N = 512
CH = 256          # n_out
NIN = 256         # n_in
NCORES = 8
RB = N // NCORES  # 64 rows per core
SCOL = 320        # rotated-column rectangle width
NSEG = 5          # 64-wide column segments
F32 = mybir.dt.float32
BF16 = mybir.dt.bfloat16
BF16NP = ml_dtypes.bfloat16


# --------------------------------------------------------------------------
# host-side input builders
# --------------------------------------------------------------------------

def _shared_inputs(W, b):
    W = np.asarray(W, np.float32)
    b = np.asarray(b, np.float32)
    wa = np.ascontiguousarray(W[:, :NIN].T)       # [f, ch]
    wb = np.ascontiguousarray(W[:, NIN:].T)
    return {
        "wa": wa.astype(BF16NP),
        "wb": wb.astype(BF16NP),
        "bcol": b.reshape(1, CH).astype(BF16NP),
    }


def _core_inputs(x, k):
    x = np.asarray(x, np.float32)
    base = RB * k
    idx = (base + np.arange(SCOL)) % N
    xr = np.ascontiguousarray(x.T[:, idx])        # [f, s] rotated
    xh = xr.astype(BF16NP)
    xl = (xr - xh.astype(np.float32)).astype(BF16NP)

    wrap = N - base
    rl = np.zeros((128, 3 * NSEG), np.float32)
    for j in range(NSEG):
        r = 1.0 if 64 * (j + 1) <= wrap else 0.0
        rl[:, j] = r
        rl[:, NSEG + j] = 1.0 - r
        rl[:, 2 * NSEG + j] = -r
    return {"xh": xh, "xl": xl, "rl": rl}


# --------------------------------------------------------------------------
# device program
# --------------------------------------------------------------------------

_PROGRAM = None


def _build_program() -> bass.Bass:
    nc = bacc.Bacc()
    f32 = F32
    AL = mybir.AluOpType

    d_xh = nc.dram_tensor("xh", [NIN, SCOL], BF16, kind="ExternalInput")
    d_xl = nc.dram_tensor("xl", [NIN, SCOL], BF16, kind="ExternalInput")
    d_wa = nc.dram_tensor("wa", [NIN, CH], BF16, kind="ExternalInput")
    d_wb = nc.dram_tensor("wb", [NIN, CH], BF16, kind="ExternalInput")
    d_bcol = nc.dram_tensor("bcol", [1, CH], BF16, kind="ExternalInput")
    d_rl = nc.dram_tensor("rl", [128, 3 * NSEG], f32, kind="ExternalInput")

    # outp[h][p, (c, t, j, s)]: ch = 128h + p, row t' = 8c + t,
    # rotated col = 64j + s.
    d_out = nc.dram_tensor("outp", [2, 128, RB * SCOL], BF16,
                           kind="ExternalOutput")

    with tile.TileContext(nc) as tc:
        with (
            tc.tile_pool(name="const", bufs=1) as cpool,
            tc.tile_pool(name="ps", bufs=4, space="PSUM") as ps,
            tc.tile_pool(name="slab0", bufs=4) as sp0,
            tc.tile_pool(name="slab1", bufs=4) as sp1,
        ):
            def load(dram, shape, dtype, tag, eng=None):
                t = cpool.tile(shape, dtype, tag=tag, name=tag)
                (eng or nc.sync).dma_start(out=t[:], in_=dram)
                return t

            xh0 = load(d_xh[0:128, :], [128, SCOL], BF16, "xh0")
            xh1 = load(d_xh[128:256, :], [128, SCOL], BF16, "xh1")
            xl0 = load(d_xl[0:128, :], [128, SCOL], BF16, "xl0", nc.scalar)
            xl1 = load(d_xl[128:256, :], [128, SCOL], BF16, "xl1", nc.scalar)
            wa0 = load(d_wa[0:128, :], [128, CH], BF16, "wa0")
            wa1 = load(d_wa[128:256, :], [128, CH], BF16, "wa1")
            wb0 = load(d_wb[0:128, :], [128, CH], BF16, "wb0", nc.scalar)
            wb1 = load(d_wb[128:256, :], [128, CH], BF16, "wb1", nc.scalar)
            bcol = load(d_bcol[:], [1, CH], BF16, "bcol")
            rlt = load(d_rl[:], [128, 3 * NSEG], f32, "rlt")

            ones = cpool.tile([1, SCOL], BF16, tag="ones", name="ones")
            nc.vector.memset(ones[:], 1.0)

            # ---- tables ----------------------------------------------------
            AT, BpT, dT, M, BSEG = {}, {}, {}, {}, {}
            for h in (0, 1):
                cs = slice(128 * h, 128 * (h + 1))
                for nm, w0, w1, with_b in (("A", wa0, wa1, False),
                                           ("B", wb0, wb1, True)):
                    p = ps.tile([128, SCOL], f32, tag="pt", name=f"pt{nm}{h}")
                    mm = nc.tensor.matmul
                    mm(p[:], w0[:, cs], xh0[:], start=True, stop=False)
                    mm(p[:], w1[:, cs], xh1[:], start=False, stop=False)
                    mm(p[:], w0[:, cs], xl0[:], start=False, stop=False)
                    mm(p[:], w1[:, cs], xl1[:], start=False,
                       stop=not with_b)
                    if with_b:
                        mm(p[:], bcol[0:1, cs], ones[:], start=False,
                           stop=True)
                    t = cpool.tile([128, SCOL], f32, tag=f"T{nm}{h}",
                                   name=f"T{nm}{h}")
                    nc.scalar.copy(out=t[:], in_=p[:])
                    (AT if nm == "A" else BpT)[h] = t

                d = cpool.tile([128, SCOL], f32, tag=f"d{h}", name=f"d{h}")
                nc.vector.tensor_sub(d[:], BpT[h][:], AT[h][:])
                dT[h] = d

                # M = AT + R*d (col term); BSEG[(j, t)] = BpT - R_j*d (bias)
                # bf16 so the big adds below run in the DVE 2x perf mode,
                # which needs innermost stride ±1 on ALL operands — hence
                # BSEG is stored duplicated in adjacent pairs [ch,(j,t,2)].
                m = cpool.tile([128, SCOL], BF16, tag=f"M{h}", name=f"M{h}")
                bs = cpool.tile([128, NSEG * RB], BF16, tag=f"BS{h}",
                                name=f"BS{h}")
                for j in range(NSEG):
                    sl = slice(64 * j, 64 * (j + 1))
                    nc.vector.scalar_tensor_tensor(
                        out=m[:, sl], in0=d[:, sl],
                        scalar=rlt[:, j:j + 1], in1=AT[h][:, sl],
                        op0=AL.mult, op1=AL.add)
                    nc.vector.scalar_tensor_tensor(
                        out=bs[:, RB * j:RB * (j + 1)], in0=d[:, 0:RB],
                        scalar=rlt[:, 2 * NSEG + j:2 * NSEG + j + 1],
                        in1=BpT[h][:, 0:RB], op0=AL.mult, op1=AL.add)
                # M8: M replicated x8 in (j, s, t8) layout -> the big TT's
                # in0 is a plain contiguous read.
                m8 = cpool.tile([128, 8 * SCOL], BF16, tag=f"M8{h}",
                                name=f"M8{h}")
                m8v = m8[:].rearrange("p (j s t) -> p j s t",
                                      j=NSEG, s=64, t=8)
                nc.vector.tensor_copy(
                    out=m8v[:, :, :, 0:1].squeeze(3),
                    in_=m[:].rearrange("p (j s) -> p j s", j=NSEG))
                for w in (1, 2, 4):
                    nc.vector.tensor_copy(out=m8v[:, :, :, w:2 * w],
                                          in_=m8v[:, :, :, 0:w])
                M[h], BSEG[h] = m8, bs

            # ---- slabs -----------------------------------------------------
            # Chunk layout (j, s, t8): t innermost means every TT operand
            # has innermost stride 1 with 3 free dims -> DVE 2x_1P mode.
            SH = [128, NSEG, 64, 8]
            for c in range(8):
                for h in (0, 1):
                    pool = sp0 if h == 0 else sp1
                    slab = pool.tile([128, 8 * SCOL], BF16, tag="sl",
                                     name=f"sl{c}_{h}")
                    out_ap = slab[:].rearrange(
                        "p (j s t) -> p j s t", j=NSEG, s=64, t=8)
                    m_ap = M[h][:].rearrange(
                        "p (j s t) -> p j s t", j=NSEG, s=64, t=8)
                    b_ap = (BSEG[h][:]
                            .rearrange("p (j t) -> p j t", j=NSEG)
                            [:, :, 8 * c:8 * (c + 1)]
                            .unsqueeze(2).broadcast_to(SH))
                    nc.vector.tensor_tensor(out_ap, m_ap, b_ap, AL.add)
                    q = nc.sync if h == 0 else nc.scalar
                    q.dma_start(
                        out=d_out[h][:, 8 * SCOL * c:8 * SCOL * (c + 1)],
                        in_=slab[:])

    nc.compile()
    return nc


def _program() -> bass.Bass:
    global _PROGRAM
    if _PROGRAM is None:
        _PROGRAM = _build_program()
    return _PROGRAM


# --------------------------------------------------------------------------
# host entry point
# --------------------------------------------------------------------------

_IDX = {}


def _band_idx(dmax):
    """(t_idx, s_idx) of rectangle entries with 1 <= s - t <= dmax."""
    if dmax not in _IDX:
        t, s = np.mgrid[0:RB, 0:SCOL]
        m = (s - t >= 1) & (s - t <= dmax)
        _IDX[dmax] = (t[m], s[m])
    return _IDX[dmax]


def _assemble(results):
    out = np.zeros((N * N, CH), np.float32)
    for k in range(NCORES):
        base = RB * k
        v = np.asarray(results[k]["outp"]).astype(np.float32)
        slab = (v.reshape(2, 128, 8, NSEG, 64, 8)
                .transpose(2, 5, 3, 4, 0, 1).reshape(RB, SCOL, CH))
        t_idx, s_idx = _band_idx(256 if k < 4 else 255)
        r_idx = base + t_idx
        j_idx = (base + s_idx) % N
        vals = slab[t_idx, s_idx]
        out[r_idx * N + j_idx] = vals
        out[j_idx * N + r_idx] = vals
    return out.reshape(N, N, CH)


def build_in_maps(x, W, b):
    shared = _shared_inputs(W, b)
    return [dict(shared, **_core_inputs(x, k)) for k in range(NCORES)]


def kernel(x, W, b):
    nc = _program()
    in_maps = build_in_maps(x, W, b)
    res = run_bass_kernel_spmd(nc, in_maps, core_ids=list(range(NCORES)))
    return _assemble(res.results)


# revision 7
# speedup vs baseline: 3.0348x; 1.0176x over previous
"""Trainium2 Bass kernel for nn_LinearEncoder (gnn_message_passing), v4.

Reference, for N=512 nodes, n_in = n_out = 256:
    i, j = triu_indices(N, k=1)
    h = concat([x[i], x[j]]) @ W.T + b        # [E, 256]
    out[i, j] = h ; out = out + out.T         # [512, 512, 256], 0 diagonal

Algebraic identity (W = [W1 | W2]):  out[i, j] = A[min] + B'[max],
    A = x @ W1.T,  B' = x @ W2.T + b,  zero diagonal.

Exploits output symmetry: each unordered pair {r, j} is computed on ONE
core as bf16 and mirrored by the host.  Row r owns circular distances
d = 1..255 (+256 for r < 256); core k (rows [64k, 64k+64)) computes the
rotated-column rectangle s in [0, 320) (cols j = (64k + s) % 512), a 20%
padded cover of its band.  10.5 MB/core out — DMA roofline ~27 us at the
observed ~410 GB/s.

Device program (partition dim = output channel):
  - PE: transposed tables AT[ch, s] = W1 @ xT_rot, BpT = W2 @ xT_rot + b
    (x in bf16; the extra rounding is dwarfed by the bf16 output).
  - mix (input rmask = R_s): M = AT + R*(BpT - AT) replicated x8 into
    (j, s, t8) layout; per-row bias BSEG[ch, (j, t)] = BpT - R_j*(BpT-AT).
  - per 8-row chunk: slab[ch, (j, s, t)] = M8 + BSEG broadcast over s —
    one DVE tensor_tensor; (j, s, t)-layout puts stride-1 bf16 pairs
    innermost on all operands, which unlocks the DVE 2x_1P perf mode.
  - sync queue streams h=0 chunks, scalar queue h=1 chunks to HBM.
"""

import os
import sys

for _p in ("/opt/trn_rl_repo", "/root/.axon_site/_ro/trn_rl_repo"):
    if os.path.isdir(_p) and _p not in sys.path:
        sys.path.insert(0, _p)

import numpy as np
import ml_dtypes

import concourse.bass as bass
import concourse.bacc as bacc
import concourse.mybir as mybir
import concourse.tile as tile
from concourse.bass_utils import run_bass_kernel_spmd

N = 512
CH = 256          # n_out
NIN = 256         # n_in
NCORES = 8
RB = N // NCORES  # 64 rows per core
SCOL = 320        # rotated-column rectangle width
NSEG = 5          # 64-wide column segments
F32 = mybir.dt.float32
BF16 = mybir.dt.bfloat16
BF16NP = ml_dtypes.bfloat16


# --------------------------------------------------------------------------
# host-side input builders
# --------------------------------------------------------------------------

def _shared_inputs(W, b):
    W = np.asarray(W, np.float32)
    b = np.asarray(b, np.float32)
    wa = np.ascontiguousarray(W[:, :NIN].T)       # [f, ch]
    wb = np.ascontiguousarray(W[:, NIN:].T)
    return {
        "wa": wa.astype(BF16NP),
        "wb": wb.astype(BF16NP),
        "bcol": b.reshape(1, CH).astype(BF16NP),
    }


def _core_inputs(x, k):
    x = np.asarray(x, np.float32)
    base = RB * k
    idx = (base + np.arange(SCOL)) % N
    xr = np.ascontiguousarray(x.T[:, idx])        # [f, s] rotated
    wrap = N - base
    seg_r = np.array([1.0 if 64 * (j + 1) <= wrap else 0.0
                      for j in range(NSEG)], np.float32)
    rmask = np.broadcast_to(np.repeat(seg_r, 64), (128, SCOL))
    return {"xh": xr.astype(BF16NP),
            "rmask": rmask.astype(BF16NP)}


# --------------------------------------------------------------------------
# device program
# --------------------------------------------------------------------------

_PROGRAM = None


def _build_program() -> bass.Bass:
    nc = bacc.Bacc()
    f32 = F32
    AL = mybir.AluOpType

    d_xh = nc.dram_tensor("xh", [NIN, SCOL], BF16, kind="ExternalInput")
    d_wa = nc.dram_tensor("wa", [NIN, CH], BF16, kind="ExternalInput")
    d_wb = nc.dram_tensor("wb", [NIN, CH], BF16, kind="ExternalInput")
    d_bcol = nc.dram_tensor("bcol", [1, CH], BF16, kind="ExternalInput")
    d_rm = nc.dram_tensor("rmask", [128, SCOL], BF16, kind="ExternalInput")

    # outp[h][p, (c, j, s, t)]: ch = 128h + p, row t' = 8c + t,
    # rotated col = 64j + s.
    d_out = nc.dram_tensor("outp", [2, 128, RB * SCOL], BF16,
                           kind="ExternalOutput")

    with tile.TileContext(nc) as tc:
        with (
            tc.tile_pool(name="const", bufs=1) as cpool,
            tc.tile_pool(name="ps", bufs=4, space="PSUM") as ps,
            tc.tile_pool(name="slab0", bufs=4) as sp0,
            tc.tile_pool(name="slab1", bufs=4) as sp1,
        ):
            def load(dram, shape, dtype, tag, eng=None):
                t = cpool.tile(shape, dtype, tag=tag, name=tag)
                (eng or nc.sync).dma_start(out=t[:], in_=dram)
                return t

            xh0 = load(d_xh[0:128, :], [128, SCOL], BF16, "xh0")
            wa0 = load(d_wa[0:128, :], [128, CH], BF16, "wa0")
            xh1 = load(d_xh[128:256, :], [128, SCOL], BF16, "xh1",
                       nc.scalar)
            wa1 = load(d_wa[128:256, :], [128, CH], BF16, "wa1", nc.scalar)
            wb0 = load(d_wb[0:128, :], [128, CH], BF16, "wb0")
            wb1 = load(d_wb[128:256, :], [128, CH], BF16, "wb1", nc.scalar)
            bcol = load(d_bcol[:], [1, CH], BF16, "bcol")
            rmt = load(d_rm[:], [128, SCOL], BF16, "rmt", nc.scalar)

            ones = cpool.tile([1, SCOL], BF16, tag="ones", name="ones")
            nc.vector.memset(ones[:], 1.0)

            M8, BSEG = {}, {}

            def prep(h):
                cs = slice(128 * h, 128 * (h + 1))
                tabs = {}
                for nm, w0, w1, with_b in (("A", wa0, wa1, False),
                                           ("B", wb0, wb1, True)):
                    p = ps.tile([128, SCOL], f32, tag="pt", name=f"pt{nm}{h}")
                    mm = nc.tensor.matmul
                    mm(p[:], w0[:, cs], xh0[:], start=True, stop=False)
                    mm(p[:], w1[:, cs], xh1[:], start=False, stop=not with_b)
                    if with_b:
                        mm(p[:], bcol[0:1, cs], ones[:], start=False,
                           stop=True)
                    t = cpool.tile([128, SCOL], f32, tag=f"T{nm}{h}",
                                   name=f"T{nm}{h}")
                    nc.scalar.copy(out=t[:], in_=p[:])
                    tabs[nm] = t
                AT, BpT = tabs["A"], tabs["B"]

                d = cpool.tile([128, SCOL], f32, tag=f"d{h}", name=f"d{h}")
                nc.vector.tensor_sub(d[:], BpT[:], AT[:])
                tm = cpool.tile([128, SCOL], f32, tag=f"tm{h}",
                                name=f"tm{h}")
                nc.vector.tensor_mul(tm[:], d[:], rmt[:])

                # M8[(j, s, t8)]: slot 0 = AT + R*d, then doubled to 8.
                m8 = cpool.tile([128, 8 * SCOL], BF16, tag=f"M8{h}",
                                name=f"M8{h}")
                m8v = m8[:].rearrange("p (j s t) -> p j s t",
                                      j=NSEG, s=64, t=8)
                js = lambda ap: ap.rearrange("p (j s) -> p j s", j=NSEG)
                nc.vector.tensor_add(m8v[:, :, :, 0:1].squeeze(3),
                                     js(tm[:]), js(AT[:]))
                for w in (1, 2, 4):
                    nc.vector.tensor_copy(out=m8v[:, :, :, w:2 * w],
                                          in_=m8v[:, :, :, 0:w])

                # BSEG[(j, t)] = BpT[t] - R_j * d[t]
                bs = cpool.tile([128, NSEG * RB], BF16, tag=f"BS{h}",
                                name=f"BS{h}")
                jt = lambda ap: (ap[:, 0:RB].unsqueeze(1)
                                 .broadcast_to([128, NSEG, RB]))
                tb = cpool.tile([128, NSEG * RB], f32, tag=f"tb{h}",
                                name=f"tb{h}")
                tbv = tb[:].rearrange("p (j t) -> p j t", j=NSEG)
                nc.vector.tensor_mul(
                    tbv, jt(d), rmt[:].rearrange("p (j t) -> p j t",
                                                 j=NSEG))
                nc.vector.tensor_sub(
                    bs[:].rearrange("p (j t) -> p j t", j=NSEG),
                    jt(BpT), tbv)
                M8[h], BSEG[h] = m8, bs

            SH = [128, NSEG, 64, 8]

            def chunk(c, h):
                pool = sp0 if h == 0 else sp1
                slab = pool.tile([128, 8 * SCOL], BF16, tag="sl",
                                 name=f"sl{c}_{h}")
                out_ap = slab[:].rearrange(
                    "p (j s t) -> p j s t", j=NSEG, s=64, t=8)
                m_ap = M8[h][:].rearrange(
                    "p (j s t) -> p j s t", j=NSEG, s=64, t=8)
                b_ap = (BSEG[h][:]
                        .rearrange("p (j t) -> p j t", j=NSEG)
                        [:, :, 8 * c:8 * (c + 1)]
                        .unsqueeze(2).broadcast_to(SH))
                nc.vector.tensor_tensor(out_ap, m_ap, b_ap, AL.add)
                q = nc.sync if h == 0 else nc.scalar
                q.dma_start(
                    out=d_out[h][:, 8 * SCOL * c:8 * SCOL * (c + 1)],
                    in_=slab[:])

            prep(0)
            chunk(0, 0)
            chunk(1, 0)
            prep(1)
            chunk(0, 1)
            for c in range(2, 8):
                chunk(c, 0)
                chunk(c - 1, 1)
            chunk(7, 1)

    nc.compile()
    return nc


def _program() -> bass.Bass:
    global _PROGRAM
    if _PROGRAM is None:
        _PROGRAM = _build_program()
    return _PROGRAM


# --------------------------------------------------------------------------
# host entry point
# --------------------------------------------------------------------------

_IDX = {}


def _band_idx(dmax):
    """(t_idx, s_idx) of rectangle entries with 1 <= s - t <= dmax."""
    if dmax not in _IDX:
        t, s = np.mgrid[0:RB, 0:SCOL]
        m = (s - t >= 1) & (s - t <= dmax)
        _IDX[dmax] = (t[m], s[m])
    return _IDX[dmax]


def _assemble(results):
    out = np.zeros((N * N, CH), np.float32)
    for k in range(NCORES):
        base = RB * k
        v = np.asarray(results[k]["outp"]).astype(np.float32)
        slab = (v.reshape(2, 128, 8, NSEG, 64, 8)
                .transpose(2, 5, 3, 4, 0, 1).reshape(RB, SCOL, CH))
        t_idx, s_idx = _band_idx(256 if k < 4 else 255)
        r_idx = base + t_idx
        j_idx = (base + s_idx) % N
        vals = slab[t_idx, s_idx]
        out[r_idx * N + j_idx] = vals
        out[j_idx * N + r_idx] = vals
    return out.reshape(N, N, CH)


def build_in_maps(x, W, b):
    shared = _shared_inputs(W, b)
    return [dict(shared, **_core_inputs(x, k)) for k in range(NCORES)]


def kernel(x, W, b):
    nc = _program()
    in_maps = build_in_maps(x, W, b)
    res = run_bass_kernel_spmd(nc, in_maps, core_ids=list(range(NCORES)))
    return _assemble(res.results)
